# revision 1
# baseline (speedup 1.0000x reference)
"""Trainium2 Bass kernel for nn_AttentionPool (gnn_message_passing).

Strategy
--------
Math restructure (exactly equivalent to the reference up to fp rounding):
  score[n,h] = context_h[n,:] @ V[:,h] + c[h]        (fold W_lin/W_att/b_lin/b_att;
               V[k,h] = sum_o W_lin[h*128+o, k] * W_att[o],  c[h] = b_lin[h*128:].W_att + b_att)
  p = exp(leaky_relu(score, 0.2))                     (skip segment-max: scores are
                                                       O(1), exp cannot overflow; softmax
                                                       is shift-invariant so result is
                                                       identical up to rounding)
  denom[cls,h]  = sum_{n: y=cls} p[n,h]
  pooled[cls,h,:] = sum_{n: y=cls} p[n,h]*context_h[n,:] / denom[cls,h]

Sharding: BY CLASS. Host argsorts context_y; core k receives every node with
y in [125k, 125k+125), padded to a fixed capacity CAP. Each core fully owns
its 125 classes -> no cross-core reduction.

Device inner loop (per 128-node tile, all heavy work on PE + DVE):
  O_wh[n, cls] = (ciota==y_rel[n]) * p[n,h]        one fused DVE op per head
  acc[cls, h*512 : h*512+257] += O_wh^T @ [H | 1]  one matmul per head (PSUM)
so per head block: cols 0:256 = weighted pool, col 256 = denom. Scores are
computed 16 tiles at a time (one PSUM bank), leaky-relu on DVE, one batched
exp on ACT per supertile. Host divides pool by denom and concatenates.
"""

import sys

sys.path.insert(0, "/opt/trn_rl_repo")

import numpy as np
import ml_dtypes

BF = ml_dtypes.bfloat16

N = 100000
INC = 256
NHEAD = 4
OUTC = 128
NCLS = 1000
NCORES = 8
CPC = NCLS // NCORES  # 125 classes per core
HB = 512  # per-head PSUM block (one bank); cols 0:256 pool, 256 denom
MW = INC + 1  # matmul moving width per tile: [H | ones]

# fixed per-core node capacity: mean 12500, std ~105 -> 13184 is ~6.5 sigma
T_TILES = 103
CAP = T_TILES * 128

_PROG_CACHE = {}
LAST_RESULT = None
LAST_PROFILE = None


def build_program(cap=CAP, t_tiles=T_TILES, s_tiles=16):
    """Build + compile the SPMD Bass program (same program on all cores)."""
    from concourse import bacc, mybir, tile

    f32 = mybir.dt.float32
    bf16 = mybir.dt.bfloat16
    AF = mybir.ActivationFunctionType
    OP = mybir.AluOpType

    nc = bacc.Bacc(
        "TRN2", target_bir_lowering=False, debug=False, num_devices=NCORES
    )

    h_nm = nc.dram_tensor("h_nm", [cap, INC], bf16, kind="ExternalInput").ap()
    h_tr = nc.dram_tensor("h_tr", [INC, cap], bf16, kind="ExternalInput").ap()
    y_rel = nc.dram_tensor("y_rel", [cap, 1], f32, kind="ExternalInput").ap()
    w_lin = nc.dram_tensor("w_lin", [NHEAD * OUTC, INC], f32, kind="ExternalInput").ap()
    b_lin_r = nc.dram_tensor("b_lin_r", [OUTC, NHEAD], f32, kind="ExternalInput").ap()
    w_att = nc.dram_tensor("w_att", [OUTC, 1], f32, kind="ExternalInput").ap()
    b_att = nc.dram_tensor("b_att", [1, 1], f32, kind="ExternalInput").ap()
    ciota = nc.dram_tensor("ciota", [128, 128], bf16, kind="ExternalInput").ap()
    out = nc.dram_tensor("out_pool", [128, NHEAD * MW], f32, kind="ExternalOutput").ap()

    nsup = (t_tiles + s_tiles - 1) // s_tiles

    def sup_range(s_):
        ts0 = s_ * s_tiles
        return ts0, min(s_tiles, t_tiles - ts0)

    with tile.TileContext(nc) as tc:
        with (
            tc.tile_pool(name="const", bufs=1) as cpool,
            tc.tile_pool(name="stream", bufs=3) as sb,
            tc.tile_pool(name="work", bufs=2) as sg,
            tc.tile_pool(name="ow", bufs=12) as owp,
            tc.tile_pool(name="ps", bufs=2, space="PSUM") as ps,
            tc.tile_pool(name="acc", bufs=1, space="PSUM") as accp,
        ):
            # ---- constants -------------------------------------------------
            ciota_sb = cpool.tile([128, 128], bf16)
            nc.sync.dma_start(out=ciota_sb[:], in_=ciota)
            watt_sb = cpool.tile([128, 1], f32)
            nc.sync.dma_start(out=watt_sb[:], in_=w_att)
            blin_sb = cpool.tile([128, NHEAD], f32)
            nc.sync.dma_start(out=blin_sb[:], in_=b_lin_r)
            batt_sb = cpool.tile([1, 1], f32)
            nc.sync.dma_start(out=batt_sb[:1], in_=b_att)
            ones_sb = cpool.tile([1, 128], bf16)
            nc.vector.memset(ones_sb[:1], 1.0)

            # ---- fold W_lin/W_att into V [256,4] (two 128-chunks), c [1,4] --
            v_bf = []
            for ch in range(2):
                v_ps = ps.tile([128, NHEAD], f32, tag="sps")
                for h in range(NHEAD):
                    wl = sg.tile([128, 128], f32, tag="wl")
                    nc.sync.dma_start(
                        out=wl[:],
                        in_=w_lin[h * 128 : (h + 1) * 128, ch * 128 : (ch + 1) * 128],
                    )
                    nc.tensor.matmul(
                        v_ps[:, h : h + 1], lhsT=wl[:], rhs=watt_sb[:],
                        start=True, stop=True,
                    )
                vb = cpool.tile([128, NHEAD], bf16, tag=f"vbf{ch}")
                nc.vector.tensor_copy(out=vb[:], in_=v_ps[:])
                v_bf.append(vb)

            c_ps = ps.tile([1, NHEAD], f32, tag="sps")
            nc.tensor.matmul(c_ps[:1], lhsT=watt_sb[:], rhs=blin_sb[:],
                             start=True, stop=True)
            c_bf = cpool.tile([1, NHEAD], bf16)
            nc.scalar.activation(c_bf[:1], c_ps[:1], AF.Identity,
                                 bias=batt_sb[:1, :1])
            c_rep = cpool.tile([1, s_tiles * NHEAD], bf16)
            for r in range(s_tiles):
                nc.vector.tensor_copy(
                    out=c_rep[:1, r * NHEAD : (r + 1) * NHEAD], in_=c_bf[:1, :]
                )

            # ---- main pipelined loop --------------------------------------
            acc = accp.tile([128, NHEAD * HB], f32)

            stream_tiles = {}
            p_tiles = {}

            def load_and_scores(s_):
                ts0, nst = sup_range(s_)
                c0 = ts0 * 128
                c1 = c0 + nst * 128
                ht0 = sb.tile([128, s_tiles * 128], bf16, tag="ht0")
                nc.sync.dma_start(out=ht0[:, : nst * 128], in_=h_tr[0:128, c0:c1])
                ht1 = sb.tile([128, s_tiles * 128], bf16, tag="ht1")
                nc.sync.dma_start(out=ht1[:, : nst * 128], in_=h_tr[128:256, c0:c1])
                hn = sb.tile([128, s_tiles * MW], bf16, tag="hn")
                hn3 = hn[:, : nst * MW].rearrange("p (j c) -> p j c", c=MW)
                nc.sync.dma_start(
                    out=hn3[:, :, 0:INC],
                    in_=h_nm[c0:c1, :].rearrange("(j p) c -> p j c", p=128),
                )
                nc.gpsimd.memset(hn3[:, :, INC : INC + 1], 1.0)
                yt = sb.tile([128, s_tiles], f32, tag="yt")
                nc.sync.dma_start(
                    out=yt[:, :nst],
                    in_=y_rel[c0:c1, 0].rearrange("(j p) -> p j", p=128),
                )
                stream_tiles[s_] = (ht0, ht1, hn, yt, nst)

                # scores for the whole supertile into one PSUM bank
                s_ps = ps.tile([128, s_tiles * NHEAD], f32, tag="sps")
                nw = nst * NHEAD
                nc.tensor.matmul(
                    s_ps[:, :nw], lhsT=ones_sb[:1], rhs=c_rep[:1, :nw],
                    start=True, stop=False, skip_group_check=True,
                )
                for j in range(nst):
                    sl = slice(j * NHEAD, (j + 1) * NHEAD)
                    nc.tensor.matmul(
                        s_ps[:, sl], lhsT=ht0[:, j * 128 : (j + 1) * 128],
                        rhs=v_bf[0][:], start=False, stop=False,
                        skip_group_check=True,
                    )
                    nc.tensor.matmul(
                        s_ps[:, sl], lhsT=ht1[:, j * 128 : (j + 1) * 128],
                        rhs=v_bf[1][:], start=False, stop=True,
                        skip_group_check=True,
                    )
                # p = exp(leaky_relu(s)); leaky(x) = max(x, 0.2x)
                t02 = sg.tile([128, s_tiles * NHEAD], f32, tag="t02")
                nc.vector.tensor_scalar_mul(t02[:, :nw], s_ps[:, :nw], 0.2)
                slr = sg.tile([128, s_tiles * NHEAD], f32, tag="slr")
                nc.vector.tensor_tensor(
                    out=slr[:, :nw], in0=s_ps[:, :nw], in1=t02[:, :nw], op=OP.max
                )
                p_sb = sg.tile([128, s_tiles * NHEAD], f32, tag="p")
                nc.scalar.activation(p_sb[:, :nw], slr[:, :nw], AF.Exp)
                p_tiles[s_] = p_sb

            def pools(s_):
                ht0, ht1, hn, yt, nst = stream_tiles.pop(s_)
                p_sb = p_tiles.pop(s_)
                ts0, _ = sup_range(s_)
                for j in range(nst):
                    t_ = ts0 + j
                    st = t_ == 0
                    sp = t_ == t_tiles - 1
                    for h in range(NHEAD):
                        o_w = owp.tile([128, 128], bf16, tag="ow")
                        nc.vector.tensor_scalar(
                            out=o_w[:], in0=ciota_sb[:],
                            scalar1=yt[:, j : j + 1],
                            scalar2=p_sb[:, j * NHEAD + h : j * NHEAD + h + 1],
                            op0=OP.is_equal, op1=OP.mult,
                        )
                        nc.tensor.matmul(
                            acc[:, h * HB : h * HB + MW],
                            lhsT=o_w[:],
                            rhs=hn[:, j * MW : (j + 1) * MW],
                            start=st, stop=sp,
                        )

            load_and_scores(0)
            for s_ in range(nsup):
                if s_ + 1 < nsup:
                    load_and_scores(s_ + 1)
                pools(s_)

            # ---- writeback -------------------------------------------------
            out_sb = cpool.tile([128, NHEAD * MW], f32)
            for h in range(NHEAD):
                dst = out_sb[:, h * MW : (h + 1) * MW]
                src = acc[:, h * HB : h * HB + MW]
                if h < 2:
                    nc.scalar.activation(dst, src, AF.Copy)
                else:
                    nc.vector.tensor_copy(out=dst, in_=src)
            nc.sync.dma_start(out=out, in_=out_sb[:])

    nc.compile()
    return nc


def _prep_inputs(context_h, W_lin, b_lin, W_att, b_att, context_y):
    """Host-side shard: argsort by class, split into 8 contiguous class
    ranges, pad each to a common per-core tile count. Returns (in_maps, T)."""
    h = np.ascontiguousarray(np.asarray(context_h, dtype=np.float32))
    y = np.asarray(context_y).astype(np.int64)
    order = np.argsort(y, kind="stable")
    ys = y[order]
    bounds = np.searchsorted(ys, np.arange(0, NCLS + 1, CPC))
    t_tiles = max(1, int(-(-int((bounds[1:] - bounds[:-1]).max()) // 128)))
    cap = t_tiles * 128

    W_lin = np.ascontiguousarray(np.asarray(W_lin, dtype=np.float32))
    b_lin_r = np.ascontiguousarray(
        np.asarray(b_lin, dtype=np.float32).reshape(NHEAD, OUTC).T
    )
    w_att = np.asarray(W_att, dtype=np.float32).reshape(OUTC, 1)
    b_att = np.asarray(b_att, dtype=np.float32).reshape(1, 1)
    ciota = np.ascontiguousarray(
        np.broadcast_to(np.arange(128, dtype=np.float32), (128, 128))
    ).astype(BF)

    in_maps = []
    for k in range(NCORES):
        lo, hi = bounds[k], bounds[k + 1]
        cnt = hi - lo
        idx = order[lo:hi]
        h_nm = np.zeros((cap, INC), dtype=BF)
        h_nm[:cnt] = h[idx].astype(BF)
        h_tr = np.zeros((INC, cap), dtype=BF)
        h_tr[:, :cnt] = np.ascontiguousarray(h[idx].T).astype(BF)
        yr = np.full((cap, 1), float(CPC), dtype=np.float32)
        yr[:cnt, 0] = (ys[lo:hi] - k * CPC).astype(np.float32)
        in_maps.append(
            {
                "h_nm": h_nm,
                "h_tr": np.ascontiguousarray(h_tr),
                "y_rel": yr,
                "w_lin": W_lin,
                "b_lin_r": b_lin_r,
                "w_att": w_att,
                "b_att": b_att,
                "ciota": ciota,
            }
        )
    return in_maps, t_tiles


def kernel(context_h, W_lin, b_lin, W_att, b_att, context_y, num_classes):
    global LAST_RESULT, LAST_PROFILE
    import os

    assert int(num_classes) == NCLS

    from concourse.bass_utils import run_bass_kernel_spmd

    in_maps, t_tiles = _prep_inputs(context_h, W_lin, b_lin, W_att, b_att, context_y)
    if t_tiles not in _PROG_CACHE:
        _PROG_CACHE[t_tiles] = build_program(cap=t_tiles * 128, t_tiles=t_tiles)
    nc = _PROG_CACHE[t_tiles]
    core_ids = list(range(NCORES))
    res = run_bass_kernel_spmd(nc, in_maps, core_ids)
    LAST_RESULT = res

    if os.environ.get("KERNEL_PROFILE") == "1":
        # separate traced run AFTER the plain one (profile-start on a cold
        # client deadlocks the axon terminal)
        LAST_PROFILE = run_bass_kernel_spmd(nc, in_maps, core_ids, trace=True)

    outp = np.empty((NCLS, NHEAD * INC), dtype=np.float32)
    for k in range(NCORES):
        o = np.asarray(res.results[k]["out_pool"])  # [128, 4*257]
        ob = o[:CPC].reshape(CPC, NHEAD, MW)
        pool = ob[:, :, :INC]
        den = ob[:, :, INC]
        den = np.where(den != 0.0, den, 1.0)
        outp[k * CPC : (k + 1) * CPC] = (pool / den[:, :, None]).reshape(
            CPC, NHEAD * INC
        )
    return outp

